# revision 1
# baseline (speedup 1.0000x reference)
"""LSTM autoencoder (2-layer enc + autoregressive 2-layer dec + fc) on 8 trn2 cores.

Sharding: pure data parallel over batch (4096 -> 512/core, 4 groups of 128).
On-chip layout: gates/hidden on partitions, batch on the free dim.
All gate activations unified to Tanh (sigmoid(x) = (tanh(x/2)+1)/2) so one ACT
op covers all gates; hidden/cell states are stored doubled (hs=2h, cs=2c) with
the compensating 0.5 factors folded into host-side block-diagonal weights.
Gate blocks are padded to 32-partition alignment (SBUF operands must start at
partition 0/32/64/96); pad rows carry zero weights so they compute zeros.
"""

import numpy as np
from contextlib import ExitStack

import concourse.bass as bass
import concourse.bacc as bacc
import concourse.tile as tile
import concourse.mybir as mybir
from concourse.bass_utils import run_bass_kernel_spmd

DT = mybir.dt.float32
AF = mybir.ActivationFunctionType
AO = mybir.AluOpType

B, T, F, H = 4096, 1024, 8, 4
NCORES = 8
BC = B // NCORES          # batch per core (512)
PB = 128                  # batch per group (partition tile)
NG = BC // PB             # groups per core (4)
S = 2                     # staggered chains
NGc = NG // S             # groups per chain (2)
TW = 16                   # timesteps per DMA/transpose window
# gate-block order [I,O,G,F] -> pytorch weight-row blocks (i=0, f=1, g=2, o=3)
# chosen so STT operand pairs share base partitions: u=(ti@0,gcp@0),
# hs=(to@32,tc@32), v=(tf@96,cs@96)
GATE_PY = [0, 3, 2, 1]

MP = 128                  # padded psum/gate rows (4 blocks x 32)
HSR = 2 * NGc * 4         # encoder hs/cs rows per chain (16)
QE = 2 * NGc * 4          # encoder rows used per gate block (16)
QD = NGc * 4              # decoder rows used per gate block (8)
RD = NGc * 4              # decoder per-layer h rows per chain (8)
YR = NGc * F              # yT rows per chain (16)


def _gslice(w, g):
    p = GATE_PY[g]
    return w[4 * p:4 * p + 4]


def build_consts(inp):
    f32 = np.float32
    eWih0, eWhh0, eb0 = inp["enc_Wih0"], inp["enc_Whh0"], inp["enc_b0"]
    eWih1, eWhh1, eb1 = inp["enc_Wih1"], inp["enc_Whh1"], inp["enc_b1"]
    dWih0, dWhh0, db0 = inp["dec_Wih0"], inp["dec_Whh0"], inp["dec_b0"]
    dWih1, dWhh1, db1 = inp["dec_Wih1"], inp["dec_Whh1"], inp["dec_b1"]
    fcW, fcb = inp["fc_W"], inp["fc_b"]

    # encoder psum row m for gate-block g, layer L, group gr, unit u
    def m_of(g, L, gr, u):
        return g * 32 + L * (NGc * 4) + gr * 4 + u

    # decoder psum row (per layer)
    def md_of(g, gr, u):
        return g * 32 + gr * 4 + u

    whc = np.zeros((HSR, MP), f32)   # rows: [h0 g0..g_(NGc-1); h1 ...] x4 units
    wxcs = [np.zeros((F, MP), f32) for _ in range(NGc)]
    asc_e = np.ones((MP, 1), f32)
    abe = np.zeros((MP, 1), f32)
    abe0 = np.zeros((MP, 1), f32)    # step-0 bias: forces layer-1 state to 0
    for g in range(4):
        w0h = _gslice(eWhh0, g)
        w1i = _gslice(eWih1, g)
        w1h = _gslice(eWhh1, g)
        w0x = _gslice(eWih0, g)
        b0g = _gslice(eb0[:, None], g)[:, 0]
        b1g = _gslice(eb1[:, None], g)[:, 0]
        sc = 0.5 if g != 2 else 1.0
        for gr in range(NGc):
            for u in range(4):
                m0 = m_of(g, 0, gr, u)
                m1 = m_of(g, 1, gr, u)
                asc_e[m0, 0] = sc
                asc_e[m1, 0] = sc
                abe[m0, 0] = sc * b0g[u]
                abe[m1, 0] = sc * b1g[u]
                abe0[m0, 0] = sc * b0g[u]
                abe0[m1, 0] = -60.0
                for uh in range(H):
                    rh0 = gr * 4 + uh
                    rh1 = NGc * 4 + gr * 4 + uh
                    whc[rh0, m0] = 0.5 * w0h[u, uh]
                    whc[rh0, m1] = 0.5 * w1i[u, uh]
                    whc[rh1, m1] = 0.5 * w1h[u, uh]
                for f in range(F):
                    wxcs[gr][f, m0] = w0x[u, f]

    wcomp = dWih0 @ (0.5 * fcW)
    bshift = dWih0 @ fcb
    wh0 = np.zeros((RD, MP), f32)
    wx0 = np.zeros((RD, MP), f32)
    wh1i = np.zeros((RD, MP), f32)
    wh1 = np.zeros((RD, MP), f32)
    ascd = np.ones((MP, 1), f32)
    abd0 = np.zeros((MP, 1), f32)
    abd0f = np.zeros((MP, 1), f32)
    abd1 = np.zeros((MP, 1), f32)
    for g in range(4):
        w0h = _gslice(dWhh0, g)
        wx = _gslice(wcomp, g)
        w1i = _gslice(dWih1, g)
        w1h = _gslice(dWhh1, g)
        b0g = _gslice(db0[:, None], g)[:, 0]
        bsg = _gslice(bshift[:, None], g)[:, 0]
        b1g = _gslice(db1[:, None], g)[:, 0]
        sc = 0.5 if g != 2 else 1.0
        for gr in range(NGc):
            for u in range(4):
                m = md_of(g, gr, u)
                ascd[m, 0] = sc
                abd0f[m, 0] = sc * b0g[u]
                abd0[m, 0] = sc * (b0g[u] + bsg[u])
                abd1[m, 0] = sc * b1g[u]
                for uh in range(H):
                    r = gr * 4 + uh
                    wh0[r, m] = 0.5 * w0h[u, uh]
                    wx0[r, m] = wx[u, uh]
                    wh1i[r, m] = 0.5 * w1i[u, uh]
                    wh1[r, m] = 0.5 * w1h[u, uh]

    wfc = np.zeros((RD, YR), f32)
    fcbv = np.zeros((YR, 1), f32)
    for gr in range(NGc):
        for f in range(F):
            fcbv[gr * F + f, 0] = fcb[f]
            for uh in range(H):
                wfc[gr * 4 + uh, gr * F + f] = 0.5 * fcW[f, uh]

    out = {
        "whc": whc, "asc_e": asc_e, "abe": abe, "abe0": abe0,
        "wh0": wh0, "wx0": wx0, "wh1i": wh1i, "wh1": wh1,
        "ascd": ascd, "abd0": abd0, "abd0f": abd0f, "abd1": abd1,
        "wfc": wfc, "fcbv": fcbv,
        "ident": np.eye(PB, dtype=f32),
    }
    for j in range(NGc):
        out[f"wxc{j}"] = wxcs[j]
    return out


def const_shapes():
    shp = {
        "whc": (HSR, MP), "asc_e": (MP, 1), "abe": (MP, 1), "abe0": (MP, 1),
        "wh0": (RD, MP), "wx0": (RD, MP), "wh1i": (RD, MP), "wh1": (RD, MP),
        "ascd": (MP, 1), "abd0": (MP, 1), "abd0f": (MP, 1), "abd1": (MP, 1),
        "wfc": (RD, YR), "fcbv": (YR, 1), "ident": (PB, PB),
    }
    for j in range(NGc):
        shp[f"wxc{j}"] = (F, MP)
    return shp


def build_nc(Tl=T):
    nc = bacc.Bacc("TRN2", target_bir_lowering=False, debug=False)
    Xd = nc.dram_tensor("x", [BC, Tl, F], DT, kind="ExternalInput")
    Yd = nc.dram_tensor("y", [BC, Tl, F], DT, kind="ExternalOutput")
    cshapes = const_shapes()
    cdram = {k: nc.dram_tensor(k, list(s), DT, kind="ExternalInput")
             for k, s in cshapes.items()}

    def gb0(c, j):
        return (c * NGc + j) * PB

    with tile.TileContext(nc) as tc, ExitStack() as ctx:
        p = lambda name, bufs, **kw: ctx.enter_context(
            tc.tile_pool(name=name, bufs=bufs, **kw))
        wsb = p("wsb", 1)
        xsp = [[p(f"xs{c}{j}", 2) for j in range(NGc)] for c in range(S)]
        psT = p("psT", 3, space="PSUM")
        psZ = p("psZ", 3, space="PSUM")
        psO = p("psO", 2, space="PSUM")
        xtp = p("xt", 4)
        tgp = p("tg", 4)
        up = p("u", 4)
        vp = p("v", 4)
        csp = p("cs", 6)
        tcp = p("tc", 4)
        gcp_pool = p("gc", 4)
        hsp = p("hs", 6)
        rhp = p("rh", 4)
        ytp = p("yt", 4)
        osb = p("osb", 4)

        csb = {}
        for k, s in cshapes.items():
            t_ = wsb.tile(list(s), DT, name=f"c_{k}")
            nc.sync.dma_start(t_[:, :], cdram[k].ap()[:, :])
            csb[k] = t_
        ident = csb["ident"]

        # ---------------- encoder ----------------
        # merged 2-layer step: layer-0 lane at time n, layer-1 lane at n-1
        hs_prev, cs_prev = [None] * S, [None] * S
        for c in range(S):
            hs_prev[c] = hsp.tile([HSR, PB], DT, name="hs")
            nc.vector.memset(hs_prev[c][:, :], 0.0)
            cs_prev[c] = csp.tile([96 + HSR, PB], DT, name="cs")
            nc.vector.memset(cs_prev[c][96:96 + HSR, :], 0.0)

        xs_cur = [[None] * NGc for _ in range(S)]
        enc_h2, enc_c2, enc_h1, enc_c1 = [None] * S, [None] * S, [None] * S, [None] * S

        for n in range(Tl + 1):
            if n < Tl and n % TW == 0:
                for c in range(S):
                    for j in range(NGc):
                        xs = xsp[c][j].tile([PB, TW * F], DT, name="xs")
                        nc.sync.dma_start(
                            xs[:, :].rearrange("p (t f) -> p t f", f=F),
                            Xd.ap()[gb0(c, j):gb0(c, j) + PB, n:n + TW, :])
                        xs_cur[c][j] = xs
            for c in range(S):
                xts = []
                if n < Tl:
                    for j in range(NGc):
                        pT = psT.tile([F, PB], DT, name="pT")
                        nc.tensor.matmul(
                            pT[:, :],
                            xs_cur[c][j][:, (n % TW) * F:(n % TW + 1) * F],
                            ident[:, :], is_transpose=True)
                        xt = xtp.tile([F, PB], DT, name="xt")
                        if j % 2 == 0:
                            nc.vector.tensor_copy(xt[:, :], pT[:, :])
                        else:
                            nc.scalar.copy(xt[:, :], pT[:, :])
                        xts.append(xt)
                pz = psZ.tile([MP, PB], DT, name="pz")
                nc.tensor.matmul(pz[:, :], csb["whc"][:, :], hs_prev[c][:, :],
                                 start=True, stop=(n == Tl))
                if n < Tl:
                    for j in range(NGc):
                        nc.tensor.matmul(pz[:, :], csb[f"wxc{j}"][:, :],
                                         xts[j][:, :], start=False,
                                         stop=(j == NGc - 1))
                tg = tgp.tile([MP, PB], DT, name="tg")
                bias = csb["abe0"] if n == 0 else csb["abe"]
                nc.scalar.activation(tg[:, :], pz[:, :], AF.Tanh,
                                     bias=bias[:, 0:1],
                                     scale=csb["asc_e"][:, 0:1])
                gc = gcp_pool.tile([QE, PB], DT, name="gc")
                nc.gpsimd.tensor_copy(gc[:, :], tg[64:64 + QE, :])
                u = up.tile([QE, PB], DT, name="u")
                nc.vector.scalar_tensor_tensor(
                    u[:, :], tg[0:QE, :], 1.0, gc[:, :], AO.add, AO.mult)
                v = vp.tile([QE, PB], DT, name="v")
                nc.vector.scalar_tensor_tensor(
                    v[:, :], tg[96:96 + QE, :], 1.0, cs_prev[c][96:96 + QE, :],
                    AO.add, AO.mult)
                csn = csp.tile([96 + QE, PB], DT, name="cs")
                nc.vector.scalar_tensor_tensor(
                    csn[96:96 + QE, :], v[:, :], 0.5, u[:, :], AO.mult, AO.add)
                tcn = tcp.tile([32 + QE, PB], DT, name="tc")
                nc.scalar.activation(tcn[32:32 + QE, :], csn[96:96 + QE, :],
                                     AF.Tanh, bias=0.0, scale=0.5)
                tow = up.tile([32 + QE, PB], DT, name="tow")
                nc.gpsimd.tensor_scalar_add(tow[32:32 + QE, :],
                                            tg[32:32 + QE, :], 1.0)
                hsn = hsp.tile([QE, PB], DT, name="hs")
                nc.gpsimd.tensor_tensor(hsn[:, :], tow[32:32 + QE, :],
                                        tcn[32:32 + QE, :], AO.mult)
                if n == Tl - 1:
                    enc_h2[c], enc_c2[c] = hsn, csn
                if n == Tl:
                    enc_h1[c], enc_c1[c] = hsn, csn
                hs_prev[c], cs_prev[c] = hsn, csn

        # ---------------- decoder ----------------
        h0p, c0p, h1p, c1p, rhpv = [None] * S, [None] * S, [None] * S, [None] * S, [None] * S
        for c in range(S):
            h0p[c] = hsp.tile([RD, PB], DT, name="hs")
            nc.sync.dma_start(h0p[c][:, :], enc_h2[c][0:RD, :])
            c0p[c] = csp.tile([96 + RD, PB], DT, name="cs")
            nc.sync.dma_start(c0p[c][96:96 + RD, :], enc_c2[c][96:96 + RD, :])
            h1p[c] = hsp.tile([RD, PB], DT, name="hs")
            nc.sync.dma_start(h1p[c][:, :], enc_h1[c][RD:2 * RD, :])
            c1p[c] = csp.tile([96 + RD, PB], DT, name="cs")
            nc.sync.dma_start(c1p[c][96:96 + RD, :],
                              enc_c1[c][96 + RD:96 + 2 * RD, :])

        psO_cur = [None] * S
        for t in range(Tl):
            if t % TW == 0:
                for c in range(S):
                    psO_cur[c] = psO.tile([PB, TW * YR], DT, name="psO")
            jblk = TW - 1 - (t % TW)
            for c in range(S):

                def cell(wh_in, rhs_in, wh_rec, h_rec, c_rec, bias_ap):
                    pz = psZ.tile([MP, PB], DT, name="pz")
                    nc.tensor.matmul(pz[:, :], wh_rec[:, :], h_rec[:, :],
                                     start=True, stop=(rhs_in is None))
                    if rhs_in is not None:
                        nc.tensor.matmul(pz[:, :], wh_in[:, :], rhs_in[:, :],
                                         start=False, stop=True)
                    tg = tgp.tile([MP, PB], DT, name="tg")
                    nc.scalar.activation(tg[:, :], pz[:, :], AF.Tanh,
                                         bias=bias_ap, scale=csb["ascd"][:, 0:1])
                    gc = gcp_pool.tile([QD, PB], DT, name="gc")
                    nc.gpsimd.tensor_copy(gc[:, :], tg[64:64 + QD, :])
                    u = up.tile([QD, PB], DT, name="u")
                    nc.vector.scalar_tensor_tensor(
                        u[:, :], tg[0:QD, :], 1.0, gc[:, :], AO.add, AO.mult)
                    v = vp.tile([QD, PB], DT, name="v")
                    nc.vector.scalar_tensor_tensor(
                        v[:, :], tg[96:96 + QD, :], 1.0, c_rec[96:96 + QD, :],
                        AO.add, AO.mult)
                    csn = csp.tile([96 + QD, PB], DT, name="cs")
                    nc.vector.scalar_tensor_tensor(
                        csn[96:96 + QD, :], v[:, :], 0.5, u[:, :],
                        AO.mult, AO.add)
                    tcn = tcp.tile([32 + QD, PB], DT, name="tc")
                    nc.scalar.activation(tcn[32:32 + QD, :], csn[96:96 + QD, :],
                                         AF.Tanh, bias=0.0, scale=0.5)
                    tow = up.tile([32 + QD, PB], DT, name="tow")
                    nc.gpsimd.tensor_scalar_add(tow[32:32 + QD, :],
                                                tg[32:32 + QD, :], 1.0)
                    hsn = hsp.tile([QD, PB], DT, name="hs")
                    nc.gpsimd.tensor_tensor(hsn[:, :], tow[32:32 + QD, :],
                                            tcn[32:32 + QD, :], AO.mult)
                    return hsn, csn

                bias0 = csb["abd0f"][:, 0:1] if t == 0 else csb["abd0"][:, 0:1]
                h0n, c0n = cell(csb["wx0"], None if t == 0 else rhpv[c],
                                csb["wh0"], h0p[c], c0p[c], bias0)
                h1n, c1n = cell(csb["wh1i"], h0n, csb["wh1"], h1p[c],
                                c1p[c], csb["abd1"][:, 0:1])
                rh = rhp.tile([RD, PB], DT, name="rh")
                nc.gpsimd.tensor_scalar_max(rh[:, :], h1n[:, :], 0.0)
                py = psT.tile([YR, PB], DT, name="pT")
                nc.tensor.matmul(py[:, :], csb["wfc"][:, :], rh[:, :],
                                 start=True, stop=True)
                yt = ytp.tile([YR, PB], DT, name="yt")
                nc.vector.tensor_scalar(yt[:, :], py[:, :], csb["fcbv"][:, 0:1],
                                        None, op0=AO.add)
                nc.tensor.matmul(psO_cur[c][:, jblk * YR:(jblk + 1) * YR],
                                 yt[:, :], ident[0:YR, 0:YR], is_transpose=True)
                h0p[c], c0p[c], h1p[c], c1p[c], rhpv[c] = h0n, c0n, h1n, c1n, rh
            if t % TW == TW - 1:
                base = Tl - TW * (t // TW + 1)
                for c in range(S):
                    src = psO_cur[c][:, :].rearrange(
                        "p (t g f) -> p t g f", g=NGc, f=F)
                    for j in range(NGc):
                        ob = osb.tile([PB, TW * F], DT, name="ob")
                        nc.vector.tensor_copy(
                            ob[:, :].rearrange("p (t f) -> p t f", f=F),
                            src[:, :, j, :])
                        nc.sync.dma_start(
                            Yd.ap()[gb0(c, j):gb0(c, j) + PB, base:base + TW, :],
                            ob[:, :].rearrange("p (t f) -> p t f", f=F))
    nc.compile()
    return nc


_NC_CACHE = {}


def get_nc(Tl=T):
    if Tl not in _NC_CACHE:
        _NC_CACHE[Tl] = build_nc(Tl)
    return _NC_CACHE[Tl]


def kernel(**inputs):
    X = np.ascontiguousarray(np.asarray(inputs["X"], dtype=np.float32))
    Tl = X.shape[1]
    consts = build_consts({k: np.asarray(v, dtype=np.float32)
                           for k, v in inputs.items() if k != "X"})
    nc = get_nc(Tl)
    in_maps = []
    for core in range(NCORES):
        m = {"x": X[core * BC:(core + 1) * BC]}
        m.update(consts)
        in_maps.append(m)
    res = run_bass_kernel_spmd(nc, in_maps, core_ids=list(range(NCORES)))
    out = np.concatenate([r["y"] for r in res.results], axis=0)
    return out.astype(np.float32)

